# revision 30
# baseline (speedup 1.0000x reference)
"""Bass/Tile MHA kernel (RoPE, causal) distributed over 8 TRN2 NeuronCores.

Sharding: tensor-parallel over 2 head-groups x data-parallel over 4 batches.
Core c handles batch c//2, head-group c%2 (8 heads). Wq/Wk/Wv split
column-wise (head channels), Wo row-wise; the partial output projection is
reduce-scattered pairwise ([[0,1],[2,3],[4,5],[6,7]]).

Device-side layout notes:
- All matmul inputs bf16; PSUM accumulation f32.
- Q/K channels permuted per-head to [evens|odds] so interleaved-pair RoPE
  becomes contiguous-half rotate; the perm cancels in Q.K^T.
- Q,K stored transposed [c, t] (channels on partitions) via direct
  transposed projection (lhsT=W^T chunk, rhs=x^T chunk).
- Scores computed transposed S^T[k, q]. Heads are processed in PAIRS
  (the two heads sharing a 128-row KT/QT chunk): the two score matmuls
  are 64-contraction row-tiles at PE positions (0,0)/(64,0) and run
  concurrently, writing the two banks of one [128,1024] PSUM tile.
- One exp (ACT) per pair [128,1024]; causal masking is applied POST-exp
  by zeroing the upper triangle of eT with gpsimd.affine_select (no
  -inf band add needed; scores/8 never overflow).
- Softmax denominator comes free from an appended ones-column in V
  (row 64 of each AV PSUM output); 1/x via reciprocal_approx_fast.
- The timing loop body is 2x phase-unrolled with double-buffered QT/KT
  so the next phase's QK projection matmuls can fill PE gaps during the
  ACT-bound attention phase (keeps the PE HAM-warm at 2.4 GHz).
"""

import numpy as np
import ml_dtypes

import concourse.bass as bass
import concourse.bacc as bacc
import concourse.mybir as mybir
import concourse.tile as tile
from concourse.bass_utils import run_bass_kernel_spmd

BF16 = ml_dtypes.bfloat16
B, S, D, H = 4, 2048, 1024, 16
G = 2            # head-groups (tensor-parallel)
HG = H // G      # heads per group = 8
HD = D // H      # 64
DG = D // G      # 512
KD = D // 128    # 8 d-chunks
C4 = DG // 128   # 4 c-chunks per group (= head pairs)
T16 = S // 128   # 16 t-chunks
TQ = S // 512    # 4 q-tiles
THETA = 10000.0
REPLICA_GROUPS = [[0, 1], [2, 3], [4, 5], [6, 7]]
N_CORES = 8

import os as _os_mod
# Collective strategy: "rs" (pairwise ReduceScatter), "full" (AllReduce),
# or "none" (debug).
COLL = _os_mod.environ.get("MHA_COLL", "rs")
# bisect switches (default = fastest config)
USE_RAF = _os_mod.environ.get("MHA_RAF", "1") == "1"    # reciprocal_approx_fast
USE_ASEL = _os_mod.environ.get("MHA_ASEL", "1") == "1"  # affine_select mask
USE_GPR = _os_mod.environ.get("MHA_GPR", "1") == "1"    # gpsimd rope ops

FP32 = mybir.dt.float32
BF = mybir.dt.bfloat16


def _emit(nc):
    xT_d = nc.dram_tensor("xt", [D, S], BF, kind="ExternalInput")
    wq_d = nc.dram_tensor("wq", [D, DG], BF, kind="ExternalInput")
    wk_d = nc.dram_tensor("wk", [D, DG], BF, kind="ExternalInput")
    wv_d = nc.dram_tensor("wv", [D, DG], BF, kind="ExternalInput")
    w2_d = nc.dram_tensor("w2", [DG, D], BF, kind="ExternalInput")
    cos_d = nc.dram_tensor("cosf", [128, S], BF, kind="ExternalInput")
    sin_d = nc.dram_tensor("sinf", [128, S], BF, kind="ExternalInput")
    out_shape = [S // 2, D] if COLL == "rs" else [S, D]
    out_d = nc.dram_tensor("out", out_shape, BF, kind="ExternalOutput")

    import os as _os
    n_loop = int(_os.environ.get("MHA_LOOP", "1"))
    import contextlib
    _sr = _os.environ.get("MHA_SR", "1") == "1"

    with tile.TileContext(nc) as tc:
        with (
            tc.tile_pool(name="persist", bufs=1) as pp,
            tc.tile_pool(name="raws", bufs=1) as rawp,
            tc.tile_pool(name="sws", bufs=1) as swp,
            tc.tile_pool(name="et", bufs=3) as etp,
            tc.tile_pool(name="nrm", bufs=1) as nrm,
            tc.tile_pool(name="fo", bufs=2) as fop,
            tc.tile_pool(name="mm", bufs=2, space="PSUM") as mmp,
            tc.tile_pool(name="sc", bufs=2, space="PSUM") as scp_p,
            tc.tile_pool(name="av", bufs=1, space="PSUM") as avp,
            tc.tile_pool(name="dram", bufs=2, space="DRAM") as dram,
        ):
            # ---------------- persistent input loads ----------------
            xT = [pp.tile([128, S], BF, name=f"xT{i}", tag=f"xT{i}") for i in range(KD)]
            wq = [pp.tile([128, DG], BF, name=f"wq{i}", tag=f"wq{i}") for i in range(KD)]
            wkk = [pp.tile([128, DG], BF, name=f"wk{i}", tag=f"wk{i}") for i in range(KD)]
            wv = [pp.tile([128, DG], BF, name=f"wv{i}", tag=f"wv{i}") for i in range(KD)]
            w2 = [pp.tile([128, D], BF, name=f"w2{i}", tag=f"w2{i}") for i in range(C4)]
            for i in range(KD):
                nc.gpsimd.dma_start(xT[i][:], xT_d[i * 128:(i + 1) * 128, :])
                nc.gpsimd.dma_start(wq[i][:], wq_d[i * 128:(i + 1) * 128, :])
                nc.gpsimd.dma_start(wkk[i][:], wk_d[i * 128:(i + 1) * 128, :])
                nc.gpsimd.dma_start(wv[i][:], wv_d[i * 128:(i + 1) * 128, :])
            for c in range(C4):
                nc.gpsimd.dma_start(w2[c][:], w2_d[c * 128:(c + 1) * 128, :])
            cos_sb = pp.tile([128, S], BF, tag="cos")
            sin_sb = pp.tile([128, S], BF, tag="sin")
            nc.gpsimd.dma_start(cos_sb[:], cos_d[:])
            nc.gpsimd.dma_start(sin_sb[:], sin_d[:])

            V = [pp.tile([128, HG * (HD + 1)], BF, name=f"V{t}", tag=f"V{t}")
                 for t in range(T16)]
            outT = [pp.tile([128, S], BF, name=f"oT{c}", tag=f"oT{c}") for c in range(C4)]
            # explicit ping-pong QT/KT buffer sets (software pipeline: the
            # QK projection for phase p+1 is emitted interleaved into phase
            # p's attention so the PE never drains)
            QTA = [pp.tile([128, S], BF, name=f"QTA{c}", tag=f"QTA{c}") for c in range(C4)]
            KTA = [pp.tile([128, S], BF, name=f"KTA{c}", tag=f"KTA{c}") for c in range(C4)]
            QTB = [pp.tile([128, S], BF, name=f"QTB{c}", tag=f"QTB{c}") for c in range(C4)]
            KTB = [pp.tile([128, S], BF, name=f"KTB{c}", tag=f"KTB{c}") for c in range(C4)]
            # softmax-denominator batch tiles: head (ch, A/B) lives at
            # partition 32*ch, columns [A: 0:512 | B: 512:1024] (engine ops
            # need 32-aligned partition bases). Unused partitions stay 1.0
            # so the batched reciprocal never sees garbage.
            denq = pp.tile([128, 1024], FP32, tag="denq")
            recq = pp.tile([128, 1024], FP32, tag="recq")
            nc.vector.memset(denq[:], 1.0)

            env = dict(
                xT=xT, wq=wq, wkk=wkk, wv=wv, w2=w2, cos_sb=cos_sb, sin_sb=sin_sb,
                V=V, outT=outT, denq=denq, recq=recq, rawp=rawp, swp=swp,
                etp=etp, nrm=nrm, fop=fop, mmp=mmp, scp_p=scp_p, avp=avp, dram=dram,
            )

            env.update(QTA=QTA, KTA=KTA, QTB=QTB, KTB=KTB)

            # ---------------- compute (optionally looped on-device) ----------------
            # Software pipeline: V and QK projections for the NEXT phase are
            # emitted interleaved into the current phase's attention
            # (per-engine instruction streams execute in emission order, so
            # overlap must be expressed by interleaved emission). A prologue
            # fills V and QT/KT set A for the first phase.
            for ti in range(T16):
                _emit_v_tile(nc, env, ti)
            for c in range(C4):
                _emit_qk_pass(nc, env, wkk, KTA[c], c)
                _emit_qk_pass(nc, env, wq, QTA[c], c)
            if n_loop > 1:
                assert n_loop % 2 == 0, "MHA_LOOP must be even (2x phase unroll)"
                loop_cm = tc.For_i(0, n_loop // 2, 1,
                                   staggered_reset=_sr,
                                   hint_engines=(mybir.EngineType.PE,
                                                 mybir.EngineType.DVE,
                                                 mybir.EngineType.Activation,
                                                 mybir.EngineType.Pool))
                with loop_cm:
                    partial = _emit_phase(nc, env, (QTA, KTA), (QTB, KTB))
                    _emit_phase(nc, env, (QTB, KTB), (QTA, KTA))
            else:
                partial = _emit_phase(nc, env, (QTA, KTA), None)
            _emit_tail(nc, dram, partial, out_d)
    nc.compile()
    return nc


def _emit_v_tile(nc, env, ti):
    """V projection for one 128-token chunk (+ appended ones column)."""
    xT, wv, V, mmp = env["xT"], env["wv"], env["V"], env["mmp"]
    p = mmp.tile([128, 512], FP32, tag="p")
    for d in range(KD):
        nc.tensor.matmul(p[:], xT[d][:, ti * 128:(ti + 1) * 128],
                         wv[d][:], start=(d == 0), stop=(d == KD - 1))
    v3 = V[ti].rearrange("p (h c) -> p h c", c=HD + 1)
    nc.vector.tensor_copy(v3[:, :, 0:HD],
                          p.rearrange("p (h c) -> p h c", c=HD))
    nc.vector.memset(v3[:, :, HD:HD + 1], 1.0)


def _emit_qk_pass(nc, env, w_sb, dst_c, c):
    """One Q-or-K projection c-pass: 32 matmuls -> cast -> swap -> RoPE."""
    xT, cos_sb, sin_sb = env["xT"], env["cos_sb"], env["sin_sb"]
    rawp, swp, mmp = env["rawp"], env["swp"], env["mmp"]
    raw = rawp.tile([128, S], BF, tag="raw")
    for tj in range(TQ):
        p = mmp.tile([128, 512], FP32, tag="p")
        for d in range(KD):
            nc.tensor.matmul(p[:], w_sb[d][:, c * 128:(c + 1) * 128],
                             xT[d][:, tj * 512:(tj + 1) * 512],
                             start=(d == 0), stop=(d == KD - 1))
        nc.vector.tensor_copy(raw[:, tj * 512:(tj + 1) * 512], p[:])
    sw = swp.tile([128, S], BF, tag="sw")
    for hh in range(2):  # 2 heads per c-chunk; 2D slices only
        o = hh * HD
        nc.gpsimd.dma_start(sw[o:o + 32, :], raw[o + 32:o + 64, :])
        nc.gpsimd.dma_start(sw[o + 32:o + 64, :], raw[o:o + 32, :])
    nc.vector.tensor_mul(dst_c[:], raw[:], cos_sb[:])
    nc.vector.tensor_mul(sw[:], sw[:], sin_sb[:])
    nc.vector.tensor_add(dst_c[:], dst_c[:], sw[:])


def _emit_phase(nc, env, cur, nxt):
    xT, wq, wkk, wv, w2 = env["xT"], env["wq"], env["wkk"], env["wv"], env["w2"]
    V, outT = env["V"], env["outT"]
    denq, recq = env["denq"], env["recq"]
    etp, nrm, fop = env["etp"], env["nrm"], env["fop"]
    mmp, scp_p, avp, dram = env["mmp"], env["scp_p"], env["avp"], env["dram"]
    QT, KT = cur

    # pending QK-projection c-passes for the NEXT phase, interleaved into
    # this phase's attention emission (fills PE gaps of the ACT-bound
    # attention, keeps the PE HAM-warm). The next phase's V projection is
    # emitted inside the LAST pair-block's ki loop: V[ti]'s final read is
    # that block's AV at ki=ti, so each V tile refills right after.
    pending = []
    if nxt is not None:
        QTn, KTn = nxt
        for c in range(C4):
            pending.append((wkk, KTn[c], c))
            pending.append((wq, QTn[c], c))

    # ---------------- attention (qi outer; head pairs = c-chunks) ----------------
    # Per qi: for each pair, score+exp+AV; av is drained UNNORMALIZED into
    # outT and its denominator row into a batch tile, so the av PSUM banks
    # free early. One reciprocal [8,512] covers all 8 heads of the qi, then
    # per-pair broadcast + in-place mul normalizes outT. The out-projection
    # for this qi's 4 row-strips follows immediately (spreads PE work).
    partial = dram.tile([S, D], BF, tag="partial")
    nblk = 0
    for qi in range(TQ):
        qs = slice(qi * 512, (qi + 1) * 512)
        nki = 4 * qi + 4
        bns = []
        for ch in range(C4):
            if nblk % 2 == 1 and pending:
                _emit_qk_pass(nc, env, *pending.pop(0))
            nblk += 1
            hA, hB = 2 * ch, 2 * ch + 1
            avA = avp.tile([HD + 1, 512], FP32, name="avA", tag="avA")
            avB = avp.tile([HD + 1, 512], FP32, name="avB", tag="avB")
            for ki in range(nki):
                scp = scp_p.tile([128, 1024], FP32, tag="scp")
                nc.tensor.matmul(
                    scp[:, 0:512], KT[ch][0:HD, ki * 128:(ki + 1) * 128],
                    QT[ch][0:HD, qs], start=True, stop=True)
                nc.tensor.matmul(
                    scp[:, 512:1024], KT[ch][HD:128, ki * 128:(ki + 1) * 128],
                    QT[ch][HD:128, qs], start=True, stop=True)
                eT = etp.tile([128, 1024], BF, tag="eT")
                j = ki - 4 * qi
                if j < 0:  # strictly below the diagonal: no masking
                    nc.scalar.activation(eT[:], scp[:],
                                         mybir.ActivationFunctionType.Exp,
                                         scale=0.125)
                else:
                    # diagonal block: cols < 128j are fully masked (zero),
                    # the 128-wide window [128j,128j+128) is triangular,
                    # cols above are fully kept. Exp only the live cols.
                    e3 = eT.rearrange("p (g f) -> p g f", g=2)
                    s3 = scp.rearrange("p (g f) -> p g f", g=2)
                    nc.scalar.activation(e3[:, :, 128 * j:512],
                                         s3[:, :, 128 * j:512],
                                         mybir.ActivationFunctionType.Exp,
                                         scale=0.125)
                    if j > 0:
                        nc.gpsimd.memset(e3[:, :, 0:128 * j], 0.0)
                    nc.gpsimd.affine_select(
                        e3[:, :, 128 * j:128 * (j + 1)],
                        e3[:, :, 128 * j:128 * (j + 1)],
                        pattern=[[0, 2], [1, 128]],
                        compare_op=mybir.AluOpType.is_ge,
                        fill=0.0,
                        base=0,
                        channel_multiplier=-1)
                nc.tensor.matmul(avA[:], V[ki][:, hA * (HD + 1):(hA + 1) * (HD + 1)],
                                 eT[:, 0:512], start=(ki == 0), stop=(ki == nki - 1))
                nc.tensor.matmul(avB[:], V[ki][:, hB * (HD + 1):(hB + 1) * (HD + 1)],
                                 eT[:, 512:1024], start=(ki == 0), stop=(ki == nki - 1))
                if nxt is not None and qi == TQ - 1 and ch == C4 - 1:
                    _emit_v_tile(nc, env, ki)  # refill V[ki] for the next phase
            # drain av: unnormalized values to bounce tiles, denominator rows
            # to the aligned batch tile (frees the av PSUM banks early)
            bnA = nrm.tile([HD, 512], BF, tag=f"bnA{ch}")
            bnB = nrm.tile([HD, 512], BF, tag=f"bnB{ch}")
            bns.append((bnA, bnB))
            nc.vector.tensor_copy(bnA[:], avA[0:HD, :])
            nc.vector.tensor_copy(bnB[:], avB[0:HD, :])
            nc.vector.tensor_copy(denq[32 * ch:32 * ch + 1, 0:512],
                                  avA[HD:HD + 1, :])
            nc.vector.tensor_copy(denq[32 * ch:32 * ch + 1, 512:1024],
                                  avB[HD:HD + 1, :])
        # 1/x as exp(-ln(x)) on ACT (DVE's InstReciprocal is 8 cyc/elem);
        # the 1.0 background of denq round-trips to 1.0
        nc.scalar.activation(recq[:], denq[:], mybir.ActivationFunctionType.Ln)
        nc.scalar.activation(denq[:], recq[:], mybir.ActivationFunctionType.Exp,
                             scale=-1.0)
        for ch in range(C4):
            stA = nrm.tile([1, 512], BF, tag="stA")
            stB = nrm.tile([1, 512], BF, tag="stB")
            nc.vector.tensor_copy(stA[:], denq[32 * ch:32 * ch + 1, 0:512])
            nc.vector.tensor_copy(stB[:], denq[32 * ch:32 * ch + 1, 512:1024])
            rrepA = nrm.tile([HD, 512], BF, tag="rrepA")
            rrepB = nrm.tile([HD, 512], BF, tag="rrepB")
            nc.gpsimd.partition_broadcast(rrepA[:], stA[:])
            nc.gpsimd.partition_broadcast(rrepB[:], stB[:])
            bnA, bnB = bns[ch]
            nc.vector.tensor_mul(outT[ch][0:HD, qs], bnA[:], rrepA[:])
            nc.vector.tensor_mul(outT[ch][HD:128, qs], bnB[:], rrepB[:])

        # ---------------- output projection for this qi's strips ----------------
        for ti in range(4 * qi, 4 * qi + 4):
            for eh in range(2):
                fp = mmp.tile([128, 512], FP32, tag="p")
                for c in range(C4):
                    nc.tensor.matmul(fp[:], outT[c][:, ti * 128:(ti + 1) * 128],
                                     w2[c][:, eh * 512:(eh + 1) * 512],
                                     start=(c == 0), stop=(c == C4 - 1))
                fo = fop.tile([128, 512], BF, tag="fo")
                if eh == 0:
                    nc.scalar.copy(fo[:], fp[:])
                else:
                    nc.vector.tensor_copy(fo[:], fp[:])
                nc.gpsimd.dma_start(
                    partial[ti * 128:(ti + 1) * 128, eh * 512:(eh + 1) * 512],
                    fo[:])
    while pending:  # any passes not consumed by the interleave cadence
        _emit_qk_pass(nc, env, *pending.pop(0))
    return partial


def _emit_tail(nc, dram, partial, out_d):
    if COLL == "full":
        arout = dram.tile([S, D], BF, tag="arout")
        nc.gpsimd.collective_compute(
            "AllReduce", mybir.AluOpType.add,
            replica_groups=REPLICA_GROUPS,
            ins=[partial.opt()],
            outs=[arout.opt()],
        )
        nc.gpsimd.dma_start(out_d[:], arout[:])
    elif COLL == "rs":
        rsout = dram.tile([S // 2, D], BF, tag="rsout")
        nc.gpsimd.collective_compute(
            "ReduceScatter", mybir.AluOpType.add,
            replica_groups=REPLICA_GROUPS,
            ins=[partial.opt()],
            outs=[rsout.opt()],
        )
        nc.gpsimd.dma_start(out_d[:], rsout[:])
    elif COLL == "none":
        nc.gpsimd.dma_start(out_d[:], partial[:])


_NC = None


def _get_nc():
    global _NC
    if _NC is None:
        _NC = _emit(bacc.Bacc("TRN2", target_bir_lowering=False, debug=False,
                              num_devices=N_CORES))
    return _NC


def _prep_in_maps(x, token_positions, Wq, Wk, Wv, Wo):
    x = np.asarray(x, np.float32)
    tp = np.asarray(token_positions)
    Wq, Wk, Wv, Wo = (np.asarray(w, np.float32) for w in (Wq, Wk, Wv, Wo))

    # per-head [evens|odds] channel perm within each group's 512 rows
    base = np.arange(HG)[:, None] * HD
    ev = np.concatenate([np.arange(0, HD, 2), np.arange(1, HD, 2)])
    perm_local = (base + ev[None, :]).reshape(-1)  # [512]

    inv = np.exp(-np.log(THETA) * np.arange(0, HD, 2, dtype=np.float64) / HD)

    gw = []
    for g in range(G):
        rows = g * DG + perm_local
        gw.append(dict(
            wq=np.ascontiguousarray(Wq[rows, :].T).astype(BF16),
            wk=np.ascontiguousarray(Wk[rows, :].T).astype(BF16),
            wv=np.ascontiguousarray(Wv[g * DG:(g + 1) * DG, :].T).astype(BF16),
            w2=np.ascontiguousarray(Wo[:, g * DG:(g + 1) * DG].T).astype(BF16),
        ))

    in_maps = []
    for core in range(N_CORES):
        b, g = core // G, core % G
        ang = tp[b].astype(np.float64)[:, None] * inv[None, :]  # [S, 32]
        cosB = np.cos(ang).T.astype(np.float32)  # [32, S]
        sinB = np.sin(ang).T.astype(np.float32)
        cosf = np.tile(cosB, (4, 1)).astype(BF16)
        sinf = np.concatenate([-sinB, sinB, -sinB, sinB], 0).astype(BF16)
        in_maps.append(dict(
            xt=np.ascontiguousarray(x[b].T).astype(BF16),
            cosf=cosf, sinf=sinf, **gw[g],
        ))
    return in_maps


def kernel(x, token_positions, Wq, Wk, Wv, Wo):
    nc = _get_nc()
    in_maps = _prep_in_maps(x, token_positions, Wq, Wk, Wv, Wo)
    res = run_bass_kernel_spmd(nc, in_maps, list(range(N_CORES)))
    if COLL == "rs":
        # each core of a pair holds half the reduced rows (rank order)
        out = np.stack([
            np.concatenate(
                [res.results[2 * b]["out"], res.results[2 * b + 1]["out"]], 0)
            for b in range(B)
        ])
    else:
        out = np.stack([res.results[2 * b]["out"] for b in range(B)])
    return np.ascontiguousarray(out.astype(np.float32))


def build_runner(in_maps):
    """Persistent jitted SPMD executable + device-resident inputs, for timing.

    Mirrors bass2jax.run_bass_via_pjrt's multi-core path, but keeps the
    compiled callable and device inputs so repeated calls measure device
    execution only (no retrace/restage).
    """
    import jax
    from jax.sharding import Mesh, PartitionSpec, NamedSharding
    try:
        from jax.experimental.shard_map import shard_map
    except ImportError:
        from jax.shard_map import shard_map
    from concourse.bass2jax import _bass_exec_p, install_neuronx_cc_hook, partition_id_tensor

    nc = _get_nc()
    install_neuronx_cc_hook()

    partition_name = nc.partition_id_tensor.name if nc.partition_id_tensor else None
    in_names, out_names, out_avals = [], [], []
    for alloc in nc.m.functions[0].allocations:
        if not isinstance(alloc, mybir.MemoryLocationSet):
            continue
        name = alloc.memorylocations[0].name
        if alloc.kind == "ExternalInput":
            if name != partition_name:
                in_names.append(name)
        elif alloc.kind == "ExternalOutput":
            out_avals.append(jax.core.ShapedArray(
                tuple(alloc.tensor_shape), mybir.dt.np(alloc.dtype)))
            out_names.append(name)
    n_params = len(in_names)
    all_in_names = list(in_names) + list(out_names)
    if partition_name is not None:
        all_in_names.append(partition_name)

    def _body(*args):
        operands = list(args)
        if partition_name is not None:
            operands.append(partition_id_tensor())
        return tuple(_bass_exec_p.bind(
            *operands,
            out_avals=tuple(out_avals),
            in_names=tuple(all_in_names),
            out_names=tuple(out_names),
            lowering_input_output_aliases=(),
            sim_require_finite=True,
            sim_require_nnan=True,
            nc=nc,
        ))

    devices = jax.devices()[:N_CORES]
    mesh = Mesh(np.asarray(devices), ("core",))
    n_out = len(out_names)
    sharded = jax.jit(
        shard_map(_body, mesh=mesh,
                  in_specs=(PartitionSpec("core"),) * (n_params + n_out),
                  out_specs=(PartitionSpec("core"),) * n_out,
                  check_rep=False),
        keep_unused=True,
    )
    sh = NamedSharding(mesh, PartitionSpec("core"))
    concat_in = [
        jax.device_put(
            np.concatenate([np.asarray(in_maps[c][k]) for c in range(N_CORES)], 0), sh)
        for k in in_names
    ]
    concat_zeros = [
        jax.device_put(
            np.zeros((N_CORES * a.shape[0], *a.shape[1:]), a.dtype), sh)
        for a in out_avals
    ]
    return sharded, concat_in + concat_zeros, out_names, out_avals


# revision 32
# speedup vs baseline: 1.0523x; 1.0523x over previous
"""Bass/Tile MHA kernel (RoPE, causal) distributed over 8 TRN2 NeuronCores.

Sharding: tensor-parallel over 2 head-groups x data-parallel over 4 batches.
Core c handles batch c//2, head-group c%2 (8 heads). Wq/Wk/Wv split
column-wise (head channels), Wo row-wise; the partial output projection is
reduce-scattered pairwise ([[0,1],[2,3],[4,5],[6,7]]).

Device-side layout notes:
- All matmul inputs bf16; PSUM accumulation f32.
- Q/K channels permuted per-head to [evens|odds] so interleaved-pair RoPE
  becomes contiguous-half rotate; the perm cancels in Q.K^T.
- Q,K stored transposed [c, t] (channels on partitions) via direct
  transposed projection (lhsT=W^T chunk, rhs=x^T chunk).
- Scores computed transposed S^T[k, q]. Heads are processed in PAIRS
  (the two heads sharing a 128-row KT/QT chunk): the two score matmuls
  are 64-contraction row-tiles at PE positions (0,0)/(64,0) and run
  concurrently, writing the two banks of one [128,1024] PSUM tile.
- One exp (ACT) per pair [128,1024]; causal masking is applied POST-exp
  by zeroing the upper triangle of eT with gpsimd.affine_select (no
  -inf band add needed; scores/8 never overflow).
- Softmax denominator comes free from an appended ones-column in V
  (row 64 of each AV PSUM output); 1/x via reciprocal_approx_fast.
- The timing loop body is 2x phase-unrolled with double-buffered QT/KT
  so the next phase's QK projection matmuls can fill PE gaps during the
  ACT-bound attention phase (keeps the PE HAM-warm at 2.4 GHz).
"""

import numpy as np
import ml_dtypes

import concourse.bass as bass
import concourse.bacc as bacc
import concourse.mybir as mybir
import concourse.tile as tile
from concourse.bass_utils import run_bass_kernel_spmd

BF16 = ml_dtypes.bfloat16
B, S, D, H = 4, 2048, 1024, 16
G = 2            # head-groups (tensor-parallel)
HG = H // G      # heads per group = 8
HD = D // H      # 64
DG = D // G      # 512
KD = D // 128    # 8 d-chunks
C4 = DG // 128   # 4 c-chunks per group (= head pairs)
T16 = S // 128   # 16 t-chunks
TQ = S // 512    # 4 q-tiles
THETA = 10000.0
REPLICA_GROUPS = [[0, 1], [2, 3], [4, 5], [6, 7]]
N_CORES = 8

import os as _os_mod
# Collective strategy: "rs" (pairwise ReduceScatter), "full" (AllReduce),
# or "none" (debug).
COLL = _os_mod.environ.get("MHA_COLL", "rs")
# bisect switches (default = fastest config)
USE_RAF = _os_mod.environ.get("MHA_RAF", "1") == "1"    # reciprocal_approx_fast
USE_ASEL = _os_mod.environ.get("MHA_ASEL", "1") == "1"  # affine_select mask
USE_GPR = _os_mod.environ.get("MHA_GPR", "1") == "1"    # gpsimd rope ops

FP32 = mybir.dt.float32
BF = mybir.dt.bfloat16


def _emit(nc):
    xT_d = nc.dram_tensor("xt", [D, S], BF, kind="ExternalInput")
    wq_d = nc.dram_tensor("wq", [D, DG], BF, kind="ExternalInput")
    wk_d = nc.dram_tensor("wk", [D, DG], BF, kind="ExternalInput")
    wv_d = nc.dram_tensor("wv", [D, DG], BF, kind="ExternalInput")
    w2_d = nc.dram_tensor("w2", [DG, D], BF, kind="ExternalInput")
    cos_d = nc.dram_tensor("cosf", [128, S], BF, kind="ExternalInput")
    sin_d = nc.dram_tensor("sinf", [128, S], BF, kind="ExternalInput")
    out_shape = [S // 2, D] if COLL == "rs" else [S, D]
    out_d = nc.dram_tensor("out", out_shape, BF, kind="ExternalOutput")

    import os as _os
    n_loop = int(_os.environ.get("MHA_LOOP", "1"))
    import contextlib
    _sr = _os.environ.get("MHA_SR", "1") == "1"

    with tile.TileContext(nc) as tc:
        with (
            tc.tile_pool(name="persist", bufs=1) as pp,
            tc.tile_pool(name="raws", bufs=1) as rawp,
            tc.tile_pool(name="sws", bufs=1) as swp,
            tc.tile_pool(name="et", bufs=3) as etp,
            tc.tile_pool(name="nrm", bufs=1) as nrm,
            tc.tile_pool(name="fo", bufs=2) as fop,
            tc.tile_pool(name="mm", bufs=2, space="PSUM") as mmp,
            tc.tile_pool(name="sc", bufs=2, space="PSUM") as scp_p,
            tc.tile_pool(name="av", bufs=1, space="PSUM") as avp,
            tc.tile_pool(name="dram", bufs=2, space="DRAM") as dram,
        ):
            # ---------------- persistent input loads ----------------
            xT = [pp.tile([128, S], BF, name=f"xT{i}", tag=f"xT{i}") for i in range(KD)]
            wq = [pp.tile([128, DG], BF, name=f"wq{i}", tag=f"wq{i}") for i in range(KD)]
            wkk = [pp.tile([128, DG], BF, name=f"wk{i}", tag=f"wk{i}") for i in range(KD)]
            wv = [pp.tile([128, DG], BF, name=f"wv{i}", tag=f"wv{i}") for i in range(KD)]
            w2 = [pp.tile([128, D], BF, name=f"w2{i}", tag=f"w2{i}") for i in range(C4)]
            for i in range(KD):
                nc.gpsimd.dma_start(xT[i][:], xT_d[i * 128:(i + 1) * 128, :])
                nc.gpsimd.dma_start(wq[i][:], wq_d[i * 128:(i + 1) * 128, :])
                nc.gpsimd.dma_start(wkk[i][:], wk_d[i * 128:(i + 1) * 128, :])
                nc.gpsimd.dma_start(wv[i][:], wv_d[i * 128:(i + 1) * 128, :])
            for c in range(C4):
                nc.gpsimd.dma_start(w2[c][:], w2_d[c * 128:(c + 1) * 128, :])
            cos_sb = pp.tile([128, S], BF, tag="cos")
            sin_sb = pp.tile([128, S], BF, tag="sin")
            nc.gpsimd.dma_start(cos_sb[:], cos_d[:])
            nc.gpsimd.dma_start(sin_sb[:], sin_d[:])

            V = [pp.tile([128, HG * (HD + 1)], BF, name=f"V{t}", tag=f"V{t}")
                 for t in range(T16)]
            outT = [pp.tile([128, S], BF, name=f"oT{c}", tag=f"oT{c}") for c in range(C4)]
            # explicit ping-pong QT/KT buffer sets (software pipeline: the
            # QK projection for phase p+1 is emitted interleaved into phase
            # p's attention so the PE never drains)
            QTA = [pp.tile([128, S], BF, name=f"QTA{c}", tag=f"QTA{c}") for c in range(C4)]
            KTA = [pp.tile([128, S], BF, name=f"KTA{c}", tag=f"KTA{c}") for c in range(C4)]
            QTB = [pp.tile([128, S], BF, name=f"QTB{c}", tag=f"QTB{c}") for c in range(C4)]
            KTB = [pp.tile([128, S], BF, name=f"KTB{c}", tag=f"KTB{c}") for c in range(C4)]
            # softmax-denominator batch tiles: head (ch, A/B) lives at
            # partition 32*ch, columns [A: 0:512 | B: 512:1024] (engine ops
            # need 32-aligned partition bases). Unused partitions stay 1.0
            # so the batched reciprocal never sees garbage.
            denq = pp.tile([128, 1024], FP32, tag="denq")
            recq = pp.tile([128, 1024], FP32, tag="recq")
            nc.vector.memset(denq[:], 1.0)

            env = dict(
                xT=xT, wq=wq, wkk=wkk, wv=wv, w2=w2, cos_sb=cos_sb, sin_sb=sin_sb,
                V=V, outT=outT, denq=denq, recq=recq, rawp=rawp, swp=swp,
                etp=etp, nrm=nrm, fop=fop, mmp=mmp, scp_p=scp_p, avp=avp, dram=dram,
            )

            env.update(QTA=QTA, KTA=KTA, QTB=QTB, KTB=KTB)

            # ---------------- compute (optionally looped on-device) ----------------
            # Software pipeline: V and QK projections for the NEXT phase are
            # emitted interleaved into the current phase's attention
            # (per-engine instruction streams execute in emission order, so
            # overlap must be expressed by interleaved emission). A prologue
            # fills V and QT/KT set A for the first phase.
            for ti in range(T16):
                _emit_v_tile(nc, env, ti)
            for c in range(C4):
                _emit_qk_pass(nc, env, wkk, KTA[c], c)
                _emit_qk_pass(nc, env, wq, QTA[c], c)
            partial0 = dram.tile([S, D], BF, tag="partial0")
            if n_loop > 1:
                assert n_loop % 2 == 0, "MHA_LOOP must be even (2x phase unroll)"
                partial1 = dram.tile([S, D], BF, tag="partial1")
                # each phase's last-qi out-projection strips are emitted at
                # the TOP of the other phase's attention (cross-phase carry;
                # for phase1 -> phase0 this crosses the loop back edge)
                carry01 = [(partial1, ti, eh)
                           for ti in range(12, 16) for eh in range(2)]
                carry10 = [(partial0, ti, eh)
                           for ti in range(12, 16) for eh in range(2)]
                loop_cm = tc.For_i(0, n_loop // 2, 1,
                                   staggered_reset=_sr,
                                   hint_engines=(mybir.EngineType.PE,
                                                 mybir.EngineType.DVE,
                                                 mybir.EngineType.Activation,
                                                 mybir.EngineType.Pool))
                with loop_cm:
                    _emit_phase(nc, env, (QTA, KTA), (QTB, KTB), partial0, carry01)
                    _emit_phase(nc, env, (QTB, KTB), (QTA, KTA), partial1, carry10)
            else:
                _emit_phase(nc, env, (QTA, KTA), None, partial0, [])
            _emit_tail(nc, dram, partial0, out_d)
    nc.compile()
    return nc


def _emit_v_tile(nc, env, ti):
    """V projection for one 128-token chunk (+ appended ones column)."""
    xT, wv, V, mmp = env["xT"], env["wv"], env["V"], env["mmp"]
    p = mmp.tile([128, 512], FP32, tag="p")
    for d in range(KD):
        nc.tensor.matmul(p[:], xT[d][:, ti * 128:(ti + 1) * 128],
                         wv[d][:], start=(d == 0), stop=(d == KD - 1))
    v3 = V[ti].rearrange("p (h c) -> p h c", c=HD + 1)
    nc.vector.tensor_copy(v3[:, :, 0:HD],
                          p.rearrange("p (h c) -> p h c", c=HD))
    nc.vector.memset(v3[:, :, HD:HD + 1], 1.0)


def _emit_qk_pass(nc, env, w_sb, dst_c, c):
    """One Q-or-K projection c-pass: 32 matmuls -> cast -> swap -> RoPE."""
    xT, cos_sb, sin_sb = env["xT"], env["cos_sb"], env["sin_sb"]
    rawp, swp, mmp = env["rawp"], env["swp"], env["mmp"]
    raw = rawp.tile([128, S], BF, tag="raw")
    for tj in range(TQ):
        p = mmp.tile([128, 512], FP32, tag="p")
        for d in range(KD):
            nc.tensor.matmul(p[:], w_sb[d][:, c * 128:(c + 1) * 128],
                             xT[d][:, tj * 512:(tj + 1) * 512],
                             start=(d == 0), stop=(d == KD - 1))
        nc.vector.tensor_copy(raw[:, tj * 512:(tj + 1) * 512], p[:])
    sw = swp.tile([128, S], BF, tag="sw")
    for hh in range(2):  # 2 heads per c-chunk; 2D slices only
        o = hh * HD
        nc.gpsimd.dma_start(sw[o:o + 32, :], raw[o + 32:o + 64, :])
        nc.gpsimd.dma_start(sw[o + 32:o + 64, :], raw[o:o + 32, :])
    nc.vector.tensor_mul(dst_c[:], raw[:], cos_sb[:])
    nc.vector.tensor_mul(sw[:], sw[:], sin_sb[:])
    nc.vector.tensor_add(dst_c[:], dst_c[:], sw[:])


def _emit_oproj_strip(nc, env, partial, ti, eh):
    """Output projection for one (128-token, 512-emb) strip of `partial`."""
    outT, w2, mmp, fop = env["outT"], env["w2"], env["mmp"], env["fop"]
    fp = mmp.tile([128, 512], FP32, tag="p")
    for c in range(C4):
        nc.tensor.matmul(fp[:], outT[c][:, ti * 128:(ti + 1) * 128],
                         w2[c][:, eh * 512:(eh + 1) * 512],
                         start=(c == 0), stop=(c == C4 - 1))
    fo = fop.tile([128, 512], BF, tag="fo")
    if eh == 0:
        nc.scalar.copy(fo[:], fp[:])
    else:
        nc.vector.tensor_copy(fo[:], fp[:])
    nc.gpsimd.dma_start(
        partial[ti * 128:(ti + 1) * 128, eh * 512:(eh + 1) * 512], fo[:])


def _emit_phase(nc, env, cur, nxt, partial, carry_in):
    xT, wq, wkk, wv, w2 = env["xT"], env["wq"], env["wkk"], env["wv"], env["w2"]
    V, outT = env["V"], env["outT"]
    denq, recq = env["denq"], env["recq"]
    etp, nrm, fop = env["etp"], env["nrm"], env["fop"]
    mmp, scp_p, avp, dram = env["mmp"], env["scp_p"], env["avp"], env["dram"]
    QT, KT = cur

    # pending QK-projection c-passes for the NEXT phase, interleaved into
    # this phase's attention emission (fills PE gaps of the ACT-bound
    # attention, keeps the PE HAM-warm). The next phase's V projection is
    # emitted inside the LAST pair-block's ki loop: V[ti]'s final read is
    # that block's AV at ki=ti, so each V tile refills right after.
    pending = []
    if nxt is not None:
        QTn, KTn = nxt
        for c in range(C4):
            pending.append((wkk, KTn[c], c))
            pending.append((wq, QTn[c], c))
    # out-projection strip queue: carry_in holds the OTHER phase's last-qi
    # strips; each qi's own strips are queued after its normalize and popped
    # two per pair-block (so PE work lands after the normalize chain cleared)
    projq = list(carry_in)

    # ---------------- attention (qi outer; head pairs = c-chunks) ----------------
    # Per qi: for each pair, score+exp+AV (AV delayed one ki so the gpsimd
    # mask latency is pipelined); av drains UNNORMALIZED into bounce tiles
    # and denominator rows into the aligned batch tile. One reciprocal
    # [128,1024] covers all 8 heads of the qi, then per-pair broadcast +
    # mul writes normalized outT.
    nblk = 0
    for qi in range(TQ):
        qs = slice(qi * 512, (qi + 1) * 512)
        nki = 4 * qi + 4
        bns = []
        for ch in range(C4):
            if nblk % 2 == 1 and pending:
                _emit_qk_pass(nc, env, *pending.pop(0))
            for _ in range(2):
                if projq:
                    _emit_oproj_strip(nc, env, *projq.pop(0))
            nblk += 1
            hA, hB = 2 * ch, 2 * ch + 1
            avA = avp.tile([HD + 1, 512], FP32, name="avA", tag="avA")
            avB = avp.tile([HD + 1, 512], FP32, name="avB", tag="avB")
            refill_v = nxt is not None and qi == TQ - 1 and ch == C4 - 1

            def emit_av(eT_, ki_):
                nc.tensor.matmul(avA[:],
                                 V[ki_][:, hA * (HD + 1):(hA + 1) * (HD + 1)],
                                 eT_[:, 0:512],
                                 start=(ki_ == 0), stop=(ki_ == nki - 1))
                nc.tensor.matmul(avB[:],
                                 V[ki_][:, hB * (HD + 1):(hB + 1) * (HD + 1)],
                                 eT_[:, 512:1024],
                                 start=(ki_ == 0), stop=(ki_ == nki - 1))
                if refill_v:
                    _emit_v_tile(nc, env, ki_)  # refill V[ki_] for next phase

            prev = None
            for ki in range(nki):
                scp = scp_p.tile([128, 1024], FP32, tag="scp")
                nc.tensor.matmul(
                    scp[:, 0:512], KT[ch][0:HD, ki * 128:(ki + 1) * 128],
                    QT[ch][0:HD, qs], start=True, stop=True)
                nc.tensor.matmul(
                    scp[:, 512:1024], KT[ch][HD:128, ki * 128:(ki + 1) * 128],
                    QT[ch][HD:128, qs], start=True, stop=True)
                eT = etp.tile([128, 1024], BF, tag="eT")
                j = ki - 4 * qi
                if j < 0:  # strictly below the diagonal: no masking
                    nc.scalar.activation(eT[:], scp[:],
                                         mybir.ActivationFunctionType.Exp,
                                         scale=0.125)
                else:
                    # diagonal block: cols < 128j are fully masked (zero),
                    # the 128-wide window [128j,128j+128) is triangular,
                    # cols above are fully kept. Exp only the live cols.
                    e3 = eT.rearrange("p (g f) -> p g f", g=2)
                    s3 = scp.rearrange("p (g f) -> p g f", g=2)
                    nc.scalar.activation(e3[:, :, 128 * j:512],
                                         s3[:, :, 128 * j:512],
                                         mybir.ActivationFunctionType.Exp,
                                         scale=0.125)
                    if j > 0:
                        nc.gpsimd.memset(e3[:, :, 0:128 * j], 0.0)
                    nc.gpsimd.affine_select(
                        e3[:, :, 128 * j:128 * (j + 1)],
                        e3[:, :, 128 * j:128 * (j + 1)],
                        pattern=[[0, 2], [1, 128]],
                        compare_op=mybir.AluOpType.is_ge,
                        fill=0.0,
                        base=0,
                        channel_multiplier=-1)
                if prev is not None:
                    emit_av(*prev)
                prev = (eT, ki)
            emit_av(*prev)
            # drain av: unnormalized values to bounce tiles, denominator rows
            # to the aligned batch tile (frees the av PSUM banks early)
            bnA = nrm.tile([HD, 512], BF, tag=f"bnA{ch}")
            bnB = nrm.tile([HD, 512], BF, tag=f"bnB{ch}")
            bns.append((bnA, bnB))
            nc.vector.tensor_copy(bnA[:], avA[0:HD, :])
            nc.vector.tensor_copy(bnB[:], avB[0:HD, :])
            nc.vector.tensor_copy(denq[32 * ch:32 * ch + 1, 0:512],
                                  avA[HD:HD + 1, :])
            nc.vector.tensor_copy(denq[32 * ch:32 * ch + 1, 512:1024],
                                  avB[HD:HD + 1, :])
        nc.vector.reciprocal(recq[:], denq[:])
        for ch in range(C4):
            stA = nrm.tile([1, 512], BF, tag="stA")
            stB = nrm.tile([1, 512], BF, tag="stB")
            nc.vector.tensor_copy(stA[:], recq[32 * ch:32 * ch + 1, 0:512])
            nc.vector.tensor_copy(stB[:], recq[32 * ch:32 * ch + 1, 512:1024])
            rrepA = nrm.tile([HD, 512], BF, tag="rrepA")
            rrepB = nrm.tile([HD, 512], BF, tag="rrepB")
            nc.gpsimd.partition_broadcast(rrepA[:], stA[:])
            nc.gpsimd.partition_broadcast(rrepB[:], stB[:])
            bnA, bnB = bns[ch]
            nc.vector.tensor_mul(outT[ch][0:HD, qs], bnA[:], rrepA[:])
            nc.vector.tensor_mul(outT[ch][HD:128, qs], bnB[:], rrepB[:])

        # queue this qi's out-projection strips (deferred into the next
        # qi's attention; the last qi's strips go to the carry)
        strips = [(partial, ti, eh)
                  for ti in range(4 * qi, 4 * qi + 4) for eh in range(2)]
        if qi < TQ - 1 or nxt is None:
            projq.extend(strips)
    while pending:  # any passes not consumed by the interleave cadence
        _emit_qk_pass(nc, env, *pending.pop(0))
    while projq:
        _emit_oproj_strip(nc, env, *projq.pop(0))


def _emit_tail(nc, dram, partial, out_d):
    if COLL == "full":
        arout = dram.tile([S, D], BF, tag="arout")
        nc.gpsimd.collective_compute(
            "AllReduce", mybir.AluOpType.add,
            replica_groups=REPLICA_GROUPS,
            ins=[partial.opt()],
            outs=[arout.opt()],
        )
        nc.gpsimd.dma_start(out_d[:], arout[:])
    elif COLL == "rs":
        rsout = dram.tile([S // 2, D], BF, tag="rsout")
        nc.gpsimd.collective_compute(
            "ReduceScatter", mybir.AluOpType.add,
            replica_groups=REPLICA_GROUPS,
            ins=[partial.opt()],
            outs=[rsout.opt()],
        )
        nc.gpsimd.dma_start(out_d[:], rsout[:])
    elif COLL == "none":
        nc.gpsimd.dma_start(out_d[:], partial[:])


_NC = None


def _get_nc():
    global _NC
    if _NC is None:
        _NC = _emit(bacc.Bacc("TRN2", target_bir_lowering=False, debug=False,
                              num_devices=N_CORES))
    return _NC


def _prep_in_maps(x, token_positions, Wq, Wk, Wv, Wo):
    x = np.asarray(x, np.float32)
    tp = np.asarray(token_positions)
    Wq, Wk, Wv, Wo = (np.asarray(w, np.float32) for w in (Wq, Wk, Wv, Wo))

    # per-head [evens|odds] channel perm within each group's 512 rows
    base = np.arange(HG)[:, None] * HD
    ev = np.concatenate([np.arange(0, HD, 2), np.arange(1, HD, 2)])
    perm_local = (base + ev[None, :]).reshape(-1)  # [512]

    inv = np.exp(-np.log(THETA) * np.arange(0, HD, 2, dtype=np.float64) / HD)

    gw = []
    for g in range(G):
        rows = g * DG + perm_local
        gw.append(dict(
            wq=np.ascontiguousarray(Wq[rows, :].T).astype(BF16),
            wk=np.ascontiguousarray(Wk[rows, :].T).astype(BF16),
            wv=np.ascontiguousarray(Wv[g * DG:(g + 1) * DG, :].T).astype(BF16),
            w2=np.ascontiguousarray(Wo[:, g * DG:(g + 1) * DG].T).astype(BF16),
        ))

    in_maps = []
    for core in range(N_CORES):
        b, g = core // G, core % G
        ang = tp[b].astype(np.float64)[:, None] * inv[None, :]  # [S, 32]
        cosB = np.cos(ang).T.astype(np.float32)  # [32, S]
        sinB = np.sin(ang).T.astype(np.float32)
        cosf = np.tile(cosB, (4, 1)).astype(BF16)
        sinf = np.concatenate([-sinB, sinB, -sinB, sinB], 0).astype(BF16)
        in_maps.append(dict(
            xt=np.ascontiguousarray(x[b].T).astype(BF16),
            cosf=cosf, sinf=sinf, **gw[g],
        ))
    return in_maps


def kernel(x, token_positions, Wq, Wk, Wv, Wo):
    nc = _get_nc()
    in_maps = _prep_in_maps(x, token_positions, Wq, Wk, Wv, Wo)
    res = run_bass_kernel_spmd(nc, in_maps, list(range(N_CORES)))
    if COLL == "rs":
        # each core of a pair holds half the reduced rows (rank order)
        out = np.stack([
            np.concatenate(
                [res.results[2 * b]["out"], res.results[2 * b + 1]["out"]], 0)
            for b in range(B)
        ])
    else:
        out = np.stack([res.results[2 * b]["out"] for b in range(B)])
    return np.ascontiguousarray(out.astype(np.float32))


def build_runner(in_maps):
    """Persistent jitted SPMD executable + device-resident inputs, for timing.

    Mirrors bass2jax.run_bass_via_pjrt's multi-core path, but keeps the
    compiled callable and device inputs so repeated calls measure device
    execution only (no retrace/restage).
    """
    import jax
    from jax.sharding import Mesh, PartitionSpec, NamedSharding
    try:
        from jax.experimental.shard_map import shard_map
    except ImportError:
        from jax.shard_map import shard_map
    from concourse.bass2jax import _bass_exec_p, install_neuronx_cc_hook, partition_id_tensor

    nc = _get_nc()
    install_neuronx_cc_hook()

    partition_name = nc.partition_id_tensor.name if nc.partition_id_tensor else None
    in_names, out_names, out_avals = [], [], []
    for alloc in nc.m.functions[0].allocations:
        if not isinstance(alloc, mybir.MemoryLocationSet):
            continue
        name = alloc.memorylocations[0].name
        if alloc.kind == "ExternalInput":
            if name != partition_name:
                in_names.append(name)
        elif alloc.kind == "ExternalOutput":
            out_avals.append(jax.core.ShapedArray(
                tuple(alloc.tensor_shape), mybir.dt.np(alloc.dtype)))
            out_names.append(name)
    n_params = len(in_names)
    all_in_names = list(in_names) + list(out_names)
    if partition_name is not None:
        all_in_names.append(partition_name)

    def _body(*args):
        operands = list(args)
        if partition_name is not None:
            operands.append(partition_id_tensor())
        return tuple(_bass_exec_p.bind(
            *operands,
            out_avals=tuple(out_avals),
            in_names=tuple(all_in_names),
            out_names=tuple(out_names),
            lowering_input_output_aliases=(),
            sim_require_finite=True,
            sim_require_nnan=True,
            nc=nc,
        ))

    devices = jax.devices()[:N_CORES]
    mesh = Mesh(np.asarray(devices), ("core",))
    n_out = len(out_names)
    sharded = jax.jit(
        shard_map(_body, mesh=mesh,
                  in_specs=(PartitionSpec("core"),) * (n_params + n_out),
                  out_specs=(PartitionSpec("core"),) * n_out,
                  check_rep=False),
        keep_unused=True,
    )
    sh = NamedSharding(mesh, PartitionSpec("core"))
    concat_in = [
        jax.device_put(
            np.concatenate([np.asarray(in_maps[c][k]) for c in range(N_CORES)], 0), sh)
        for k in in_names
    ]
    concat_zeros = [
        jax.device_put(
            np.zeros((N_CORES * a.shape[0], *a.shape[1:]), a.dtype), sh)
        for a in out_avals
    ]
    return sharded, concat_in + concat_zeros, out_names, out_avals


# revision 33
# speedup vs baseline: 1.1215x; 1.0658x over previous
"""Bass/Tile MHA kernel (RoPE, causal) distributed over 8 TRN2 NeuronCores.

Sharding: tensor-parallel over 2 head-groups x data-parallel over 4 batches.
Core c handles batch c//2, head-group c%2 (8 heads). Wq/Wk/Wv split
column-wise (head channels), Wo row-wise; the partial output projection is
reduce-scattered pairwise ([[0,1],[2,3],[4,5],[6,7]]).

Device-side layout notes:
- All matmul inputs bf16; PSUM accumulation f32.
- Q/K channels permuted per-head to [evens|odds] so interleaved-pair RoPE
  becomes contiguous-half rotate; the perm cancels in Q.K^T.
- Q,K stored transposed [c, t] (channels on partitions) via direct
  transposed projection (lhsT=W^T chunk, rhs=x^T chunk).
- Scores computed transposed S^T[k, q]. Heads are processed in PAIRS
  (the two heads sharing a 128-row KT/QT chunk): the two score matmuls
  are 64-contraction row-tiles at PE positions (0,0)/(64,0) and run
  concurrently, writing the two banks of one [128,1024] PSUM tile.
- One exp (ACT) per pair [128,1024]; causal masking is applied POST-exp
  by zeroing the upper triangle of eT with gpsimd.affine_select (no
  -inf band add needed; scores/8 never overflow).
- Softmax denominator comes free from an appended ones-column in V
  (row 64 of each AV PSUM output); 1/x via reciprocal_approx_fast.
- The timing loop body is 2x phase-unrolled with double-buffered QT/KT
  so the next phase's QK projection matmuls can fill PE gaps during the
  ACT-bound attention phase (keeps the PE HAM-warm at 2.4 GHz).
"""

import numpy as np
import ml_dtypes

import concourse.bass as bass
import concourse.bacc as bacc
import concourse.mybir as mybir
import concourse.tile as tile
from concourse.bass_utils import run_bass_kernel_spmd

BF16 = ml_dtypes.bfloat16
B, S, D, H = 4, 2048, 1024, 16
G = 2            # head-groups (tensor-parallel)
HG = H // G      # heads per group = 8
HD = D // H      # 64
DG = D // G      # 512
KD = D // 128    # 8 d-chunks
C4 = DG // 128   # 4 c-chunks per group (= head pairs)
T16 = S // 128   # 16 t-chunks
TQ = S // 512    # 4 q-tiles
THETA = 10000.0
REPLICA_GROUPS = [[0, 1], [2, 3], [4, 5], [6, 7]]
N_CORES = 8

import os as _os_mod
# Collective strategy: "rs" (pairwise ReduceScatter), "full" (AllReduce),
# or "none" (debug).
COLL = _os_mod.environ.get("MHA_COLL", "rs")
# bisect switches (default = fastest config)
USE_RAF = _os_mod.environ.get("MHA_RAF", "1") == "1"    # reciprocal_approx_fast
USE_ASEL = _os_mod.environ.get("MHA_ASEL", "1") == "1"  # affine_select mask
USE_GPR = _os_mod.environ.get("MHA_GPR", "1") == "1"    # gpsimd rope ops

FP32 = mybir.dt.float32
BF = mybir.dt.bfloat16


def _emit(nc):
    xT_d = nc.dram_tensor("xt", [D, S], BF, kind="ExternalInput")
    wq_d = nc.dram_tensor("wq", [D, DG], BF, kind="ExternalInput")
    wk_d = nc.dram_tensor("wk", [D, DG], BF, kind="ExternalInput")
    wv_d = nc.dram_tensor("wv", [D, DG], BF, kind="ExternalInput")
    w2_d = nc.dram_tensor("w2", [DG, D], BF, kind="ExternalInput")
    cos_d = nc.dram_tensor("cosf", [128, S], BF, kind="ExternalInput")
    sin_d = nc.dram_tensor("sinf", [128, S], BF, kind="ExternalInput")
    out_shape = [S // 2, D] if COLL == "rs" else [S, D]
    out_d = nc.dram_tensor("out", out_shape, BF, kind="ExternalOutput")

    import os as _os
    n_loop = int(_os.environ.get("MHA_LOOP", "1"))
    import contextlib
    _sr = _os.environ.get("MHA_SR", "1") == "1"

    with tile.TileContext(nc) as tc:
        with (
            tc.tile_pool(name="persist", bufs=1) as pp,
            tc.tile_pool(name="raws", bufs=1) as rawp,
            tc.tile_pool(name="sws", bufs=1) as swp,
            tc.tile_pool(name="et", bufs=3) as etp,
            tc.tile_pool(name="nrm", bufs=1) as nrm,
            tc.tile_pool(name="fo", bufs=2) as fop,
            tc.tile_pool(name="mm", bufs=2, space="PSUM") as mmp,
            tc.tile_pool(name="sc", bufs=2, space="PSUM") as scp_p,
            tc.tile_pool(name="av", bufs=1, space="PSUM") as avp,
            tc.tile_pool(name="dram", bufs=2, space="DRAM") as dram,
        ):
            # ---------------- persistent input loads ----------------
            xT = [pp.tile([128, S], BF, name=f"xT{i}", tag=f"xT{i}") for i in range(KD)]
            wq = [pp.tile([128, DG], BF, name=f"wq{i}", tag=f"wq{i}") for i in range(KD)]
            wkk = [pp.tile([128, DG], BF, name=f"wk{i}", tag=f"wk{i}") for i in range(KD)]
            wv = [pp.tile([128, DG], BF, name=f"wv{i}", tag=f"wv{i}") for i in range(KD)]
            w2 = [pp.tile([128, D], BF, name=f"w2{i}", tag=f"w2{i}") for i in range(C4)]
            for i in range(KD):
                nc.gpsimd.dma_start(xT[i][:], xT_d[i * 128:(i + 1) * 128, :])
                nc.gpsimd.dma_start(wq[i][:], wq_d[i * 128:(i + 1) * 128, :])
                nc.gpsimd.dma_start(wkk[i][:], wk_d[i * 128:(i + 1) * 128, :])
                nc.gpsimd.dma_start(wv[i][:], wv_d[i * 128:(i + 1) * 128, :])
            for c in range(C4):
                nc.gpsimd.dma_start(w2[c][:], w2_d[c * 128:(c + 1) * 128, :])
            cos_sb = pp.tile([128, S], BF, tag="cos")
            sin_sb = pp.tile([128, S], BF, tag="sin")
            nc.gpsimd.dma_start(cos_sb[:], cos_d[:])
            nc.gpsimd.dma_start(sin_sb[:], sin_d[:])

            V = [pp.tile([128, HG * (HD + 1)], BF, name=f"V{t}", tag=f"V{t}")
                 for t in range(T16)]
            outT = [pp.tile([128, S], BF, name=f"oT{c}", tag=f"oT{c}") for c in range(C4)]
            # explicit ping-pong QT/KT buffer sets (software pipeline: the
            # QK projection for phase p+1 is emitted interleaved into phase
            # p's attention so the PE never drains)
            QTA = [pp.tile([128, S], BF, name=f"QTA{c}", tag=f"QTA{c}") for c in range(C4)]
            KTA = [pp.tile([128, S], BF, name=f"KTA{c}", tag=f"KTA{c}") for c in range(C4)]
            QTB = [pp.tile([128, S], BF, name=f"QTB{c}", tag=f"QTB{c}") for c in range(C4)]
            KTB = [pp.tile([128, S], BF, name=f"KTB{c}", tag=f"KTB{c}") for c in range(C4)]
            # softmax-denominator batch tiles: head (ch, A/B) lives at
            # partition 32*ch, columns [A: 0:512 | B: 512:1024] (engine ops
            # need 32-aligned partition bases). Unused partitions stay 1.0
            # so the batched reciprocal never sees garbage.
            denqA = pp.tile([128, 512], FP32, tag="denqA")
            denqB = pp.tile([128, 512], FP32, tag="denqB")
            recqA = pp.tile([128, 512], FP32, tag="recqA")
            recqB = pp.tile([128, 512], FP32, tag="recqB")
            nc.vector.memset(denqA[:], 1.0)
            nc.vector.memset(denqB[:], 1.0)

            env = dict(
                xT=xT, wq=wq, wkk=wkk, wv=wv, w2=w2, cos_sb=cos_sb, sin_sb=sin_sb,
                V=V, outT=outT, denqA=denqA, denqB=denqB, recqA=recqA,
                recqB=recqB, rawp=rawp, swp=swp,
                etp=etp, nrm=nrm, fop=fop, mmp=mmp, scp_p=scp_p, avp=avp, dram=dram,
            )

            env.update(QTA=QTA, KTA=KTA, QTB=QTB, KTB=KTB)

            # ---------------- compute (optionally looped on-device) ----------------
            # Software pipeline: V and QK projections for the NEXT phase are
            # emitted interleaved into the current phase's attention
            # (per-engine instruction streams execute in emission order, so
            # overlap must be expressed by interleaved emission). A prologue
            # fills V and QT/KT set A for the first phase.
            for ti in range(T16):
                _emit_v_tile(nc, env, ti)
            for c in range(C4):
                _emit_qk_pass(nc, env, wkk, KTA[c], c)
                _emit_qk_pass(nc, env, wq, QTA[c], c)
            partial0 = dram.tile([S, D], BF, tag="partial0")
            if n_loop > 1:
                assert n_loop % 2 == 0, "MHA_LOOP must be even (2x phase unroll)"
                partial1 = dram.tile([S, D], BF, tag="partial1")
                # each phase's last-qi out-projection strips are emitted at
                # the TOP of the other phase's attention (cross-phase carry;
                # for phase1 -> phase0 this crosses the loop back edge)
                carry01 = [(partial1, ti, eh)
                           for ti in range(12, 16) for eh in range(2)]
                carry10 = [(partial0, ti, eh)
                           for ti in range(12, 16) for eh in range(2)]
                loop_cm = tc.For_i(0, n_loop // 2, 1,
                                   staggered_reset=_sr,
                                   hint_engines=(mybir.EngineType.PE,
                                                 mybir.EngineType.DVE,
                                                 mybir.EngineType.Activation,
                                                 mybir.EngineType.Pool))
                with loop_cm:
                    _emit_phase(nc, env, (QTA, KTA), (QTB, KTB), partial0, carry01)
                    _emit_phase(nc, env, (QTB, KTB), (QTA, KTA), partial1, carry10)
            else:
                _emit_phase(nc, env, (QTA, KTA), None, partial0, [])
            _emit_tail(nc, dram, partial0, out_d)
    nc.compile()
    return nc


def _emit_v_tile(nc, env, ti):
    """V projection for one 128-token chunk (+ appended ones column)."""
    xT, wv, V, mmp = env["xT"], env["wv"], env["V"], env["mmp"]
    p = mmp.tile([128, 512], FP32, tag="p")
    for d in range(KD):
        nc.tensor.matmul(p[:], xT[d][:, ti * 128:(ti + 1) * 128],
                         wv[d][:], start=(d == 0), stop=(d == KD - 1))
    v3 = V[ti].rearrange("p (h c) -> p h c", c=HD + 1)
    nc.vector.tensor_copy(v3[:, :, 0:HD],
                          p.rearrange("p (h c) -> p h c", c=HD))
    nc.vector.memset(v3[:, :, HD:HD + 1], 1.0)


def _emit_qk_pass(nc, env, w_sb, dst_c, c):
    """One Q-or-K projection c-pass: 32 matmuls -> cast -> swap -> RoPE."""
    xT, cos_sb, sin_sb = env["xT"], env["cos_sb"], env["sin_sb"]
    rawp, swp, mmp = env["rawp"], env["swp"], env["mmp"]
    raw = rawp.tile([128, S], BF, tag="raw")
    for tj in range(TQ):
        p = mmp.tile([128, 512], FP32, tag="p")
        for d in range(KD):
            nc.tensor.matmul(p[:], w_sb[d][:, c * 128:(c + 1) * 128],
                             xT[d][:, tj * 512:(tj + 1) * 512],
                             start=(d == 0), stop=(d == KD - 1))
        nc.vector.tensor_copy(raw[:, tj * 512:(tj + 1) * 512], p[:])
    sw = swp.tile([128, S], BF, tag="sw")
    for hh in range(2):  # 2 heads per c-chunk; 2D slices only
        o = hh * HD
        nc.gpsimd.dma_start(sw[o:o + 32, :], raw[o + 32:o + 64, :])
        nc.gpsimd.dma_start(sw[o + 32:o + 64, :], raw[o:o + 32, :])
    nc.vector.tensor_mul(dst_c[:], raw[:], cos_sb[:])
    nc.vector.tensor_mul(sw[:], sw[:], sin_sb[:])
    nc.vector.tensor_add(dst_c[:], dst_c[:], sw[:])


def _emit_oproj_strip(nc, env, partial, ti, eh):
    """Output projection for one (128-token, 512-emb) strip of `partial`."""
    outT, w2, mmp, fop = env["outT"], env["w2"], env["mmp"], env["fop"]
    fp = mmp.tile([128, 512], FP32, tag="p")
    for c in range(C4):
        nc.tensor.matmul(fp[:], outT[c][:, ti * 128:(ti + 1) * 128],
                         w2[c][:, eh * 512:(eh + 1) * 512],
                         start=(c == 0), stop=(c == C4 - 1))
    fo = fop.tile([128, 512], BF, tag="fo")
    if eh == 0:
        nc.scalar.copy(fo[:], fp[:])
    else:
        nc.vector.tensor_copy(fo[:], fp[:])
    nc.gpsimd.dma_start(
        partial[ti * 128:(ti + 1) * 128, eh * 512:(eh + 1) * 512], fo[:])


def _emit_phase(nc, env, cur, nxt, partial, carry_in):
    xT, wq, wkk, wv, w2 = env["xT"], env["wq"], env["wkk"], env["wv"], env["w2"]
    V, outT = env["V"], env["outT"]
    denqA, denqB = env["denqA"], env["denqB"]
    recqA, recqB = env["recqA"], env["recqB"]
    etp, nrm, fop = env["etp"], env["nrm"], env["fop"]
    mmp, scp_p, avp, dram = env["mmp"], env["scp_p"], env["avp"], env["dram"]
    QT, KT = cur

    # pending QK-projection c-passes for the NEXT phase, interleaved into
    # this phase's attention emission (fills PE gaps of the ACT-bound
    # attention, keeps the PE HAM-warm). The next phase's V projection is
    # emitted inside the LAST pair-block's ki loop: V[ti]'s final read is
    # that block's AV at ki=ti, so each V tile refills right after.
    pending = []
    if nxt is not None:
        QTn, KTn = nxt
        for c in range(C4):
            pending.append((wkk, KTn[c], c))
            pending.append((wq, QTn[c], c))
    # out-projection strip queue: carry_in holds the OTHER phase's last-qi
    # strips; each qi's own strips are queued after its normalize and popped
    # two per pair-block (so PE work lands after the normalize chain cleared)
    projq = list(carry_in)

    # ---------------- attention (qi outer; head pairs = c-chunks) ----------------
    # Per qi: for each pair, score+exp+AV (AV delayed one ki so the gpsimd
    # mask latency is pipelined); av drains UNNORMALIZED into bounce tiles
    # and denominator rows into the aligned batch tile. One reciprocal
    # [128,1024] covers all 8 heads of the qi, then per-pair broadcast +
    # mul writes normalized outT.
    nblk = 0
    for qi in range(TQ):
        qs = slice(qi * 512, (qi + 1) * 512)
        nki = 4 * qi + 4
        bns = []
        for ch in range(C4):
            if nblk % 2 == 1 and pending:
                _emit_qk_pass(nc, env, *pending.pop(0))
            nblk += 1
            hA, hB = 2 * ch, 2 * ch + 1
            avA = avp.tile([HD + 1, 512], FP32, name="avA", tag="avA")
            avB = avp.tile([HD + 1, 512], FP32, name="avB", tag="avB")
            refill_v = nxt is not None and qi == TQ - 1 and ch == C4 - 1

            def emit_av(eT_, ki_):
                nc.tensor.matmul(avA[:],
                                 V[ki_][:, hA * (HD + 1):(hA + 1) * (HD + 1)],
                                 eT_[:, 0:512],
                                 start=(ki_ == 0), stop=(ki_ == nki - 1))
                nc.tensor.matmul(avB[:],
                                 V[ki_][:, hB * (HD + 1):(hB + 1) * (HD + 1)],
                                 eT_[:, 512:1024],
                                 start=(ki_ == 0), stop=(ki_ == nki - 1))
                if refill_v:
                    _emit_v_tile(nc, env, ki_)  # refill V[ki_] for next phase

            prev = None
            for ki in range(nki):
                scp = scp_p.tile([128, 1024], FP32, tag="scp")
                nc.tensor.matmul(
                    scp[:, 0:512], KT[ch][0:HD, ki * 128:(ki + 1) * 128],
                    QT[ch][0:HD, qs], start=True, stop=True)
                nc.tensor.matmul(
                    scp[:, 512:1024], KT[ch][HD:128, ki * 128:(ki + 1) * 128],
                    QT[ch][HD:128, qs], start=True, stop=True)
                eT = etp.tile([128, 1024], BF, tag="eT")
                j = ki - 4 * qi
                if j < 0:  # strictly below the diagonal: no masking
                    nc.scalar.activation(eT[:], scp[:],
                                         mybir.ActivationFunctionType.Exp,
                                         scale=0.125)
                else:
                    # diagonal block: cols < 128j are fully masked (zero),
                    # the 128-wide window [128j,128j+128) is triangular,
                    # cols above are fully kept. Exp only the live cols.
                    e3 = eT.rearrange("p (g f) -> p g f", g=2)
                    s3 = scp.rearrange("p (g f) -> p g f", g=2)
                    nc.scalar.activation(e3[:, :, 128 * j:512],
                                         s3[:, :, 128 * j:512],
                                         mybir.ActivationFunctionType.Exp,
                                         scale=0.125)
                    if j > 0:
                        nc.gpsimd.memset(e3[:, :, 0:128 * j], 0.0)
                    nc.gpsimd.affine_select(
                        e3[:, :, 128 * j:128 * (j + 1)],
                        e3[:, :, 128 * j:128 * (j + 1)],
                        pattern=[[0, 2], [1, 128]],
                        compare_op=mybir.AluOpType.is_ge,
                        fill=0.0,
                        base=0,
                        channel_multiplier=-1)
                if prev is not None:
                    emit_av(*prev)
                prev = (eT, ki)
            emit_av(*prev)
            # drain av: unnormalized values to bounce tiles, denominator rows
            # to the aligned batch tile (frees the av PSUM banks early)
            bnA = nrm.tile([HD, 512], BF, tag=f"bnA{ch}")
            bnB = nrm.tile([HD, 512], BF, tag=f"bnB{ch}")
            bns.append((bnA, bnB))
            nc.vector.tensor_copy(bnA[:], avA[0:HD, :])
            nc.vector.tensor_copy(bnB[:], avB[0:HD, :])
            nc.vector.tensor_copy(denqA[32 * ch:32 * ch + 1, :],
                                  avA[HD:HD + 1, :])
            nc.vector.tensor_copy(denqB[32 * ch:32 * ch + 1, :],
                                  avB[HD:HD + 1, :])
            for _ in range(2):  # deferred out-proj strips (prev qi / carry)
                if projq:
                    _emit_oproj_strip(nc, env, *projq.pop(0))
        nc.vector.reciprocal(recqA[:], denqA[:])
        nc.vector.reciprocal(recqB[:], denqB[:])
        for ch in range(C4):
            stA = nrm.tile([1, 512], BF, tag="stA")
            stB = nrm.tile([1, 512], BF, tag="stB")
            nc.vector.tensor_copy(stA[:], recqA[32 * ch:32 * ch + 1, :])
            nc.vector.tensor_copy(stB[:], recqB[32 * ch:32 * ch + 1, :])
            rrepA = nrm.tile([HD, 512], BF, tag="rrepA")
            rrepB = nrm.tile([HD, 512], BF, tag="rrepB")
            nc.gpsimd.partition_broadcast(rrepA[:], stA[:])
            nc.gpsimd.partition_broadcast(rrepB[:], stB[:])
            bnA, bnB = bns[ch]
            nc.vector.tensor_mul(outT[ch][0:HD, qs], bnA[:], rrepA[:])
            nc.vector.tensor_mul(outT[ch][HD:128, qs], bnB[:], rrepB[:])

        # queue this qi's out-projection strips (deferred into the next
        # qi's attention; the last qi's strips go to the carry)
        strips = [(partial, ti, eh)
                  for ti in range(4 * qi, 4 * qi + 4) for eh in range(2)]
        if qi < TQ - 1 or nxt is None:
            projq.extend(strips)
    while pending:  # any passes not consumed by the interleave cadence
        _emit_qk_pass(nc, env, *pending.pop(0))
    while projq:
        _emit_oproj_strip(nc, env, *projq.pop(0))


def _emit_tail(nc, dram, partial, out_d):
    if COLL == "full":
        arout = dram.tile([S, D], BF, tag="arout")
        nc.gpsimd.collective_compute(
            "AllReduce", mybir.AluOpType.add,
            replica_groups=REPLICA_GROUPS,
            ins=[partial.opt()],
            outs=[arout.opt()],
        )
        nc.gpsimd.dma_start(out_d[:], arout[:])
    elif COLL == "rs":
        rsout = dram.tile([S // 2, D], BF, tag="rsout")
        nc.gpsimd.collective_compute(
            "ReduceScatter", mybir.AluOpType.add,
            replica_groups=REPLICA_GROUPS,
            ins=[partial.opt()],
            outs=[rsout.opt()],
        )
        nc.gpsimd.dma_start(out_d[:], rsout[:])
    elif COLL == "none":
        nc.gpsimd.dma_start(out_d[:], partial[:])


_NC = None


def _get_nc():
    global _NC
    if _NC is None:
        _NC = _emit(bacc.Bacc("TRN2", target_bir_lowering=False, debug=False,
                              num_devices=N_CORES))
    return _NC


def _prep_in_maps(x, token_positions, Wq, Wk, Wv, Wo):
    x = np.asarray(x, np.float32)
    tp = np.asarray(token_positions)
    Wq, Wk, Wv, Wo = (np.asarray(w, np.float32) for w in (Wq, Wk, Wv, Wo))

    # per-head [evens|odds] channel perm within each group's 512 rows
    base = np.arange(HG)[:, None] * HD
    ev = np.concatenate([np.arange(0, HD, 2), np.arange(1, HD, 2)])
    perm_local = (base + ev[None, :]).reshape(-1)  # [512]

    inv = np.exp(-np.log(THETA) * np.arange(0, HD, 2, dtype=np.float64) / HD)

    gw = []
    for g in range(G):
        rows = g * DG + perm_local
        gw.append(dict(
            wq=np.ascontiguousarray(Wq[rows, :].T).astype(BF16),
            wk=np.ascontiguousarray(Wk[rows, :].T).astype(BF16),
            wv=np.ascontiguousarray(Wv[g * DG:(g + 1) * DG, :].T).astype(BF16),
            w2=np.ascontiguousarray(Wo[:, g * DG:(g + 1) * DG].T).astype(BF16),
        ))

    in_maps = []
    for core in range(N_CORES):
        b, g = core // G, core % G
        ang = tp[b].astype(np.float64)[:, None] * inv[None, :]  # [S, 32]
        cosB = np.cos(ang).T.astype(np.float32)  # [32, S]
        sinB = np.sin(ang).T.astype(np.float32)
        cosf = np.tile(cosB, (4, 1)).astype(BF16)
        sinf = np.concatenate([-sinB, sinB, -sinB, sinB], 0).astype(BF16)
        in_maps.append(dict(
            xt=np.ascontiguousarray(x[b].T).astype(BF16),
            cosf=cosf, sinf=sinf, **gw[g],
        ))
    return in_maps


def kernel(x, token_positions, Wq, Wk, Wv, Wo):
    nc = _get_nc()
    in_maps = _prep_in_maps(x, token_positions, Wq, Wk, Wv, Wo)
    res = run_bass_kernel_spmd(nc, in_maps, list(range(N_CORES)))
    if COLL == "rs":
        # each core of a pair holds half the reduced rows (rank order)
        out = np.stack([
            np.concatenate(
                [res.results[2 * b]["out"], res.results[2 * b + 1]["out"]], 0)
            for b in range(B)
        ])
    else:
        out = np.stack([res.results[2 * b]["out"] for b in range(B)])
    return np.ascontiguousarray(out.astype(np.float32))


def build_runner(in_maps):
    """Persistent jitted SPMD executable + device-resident inputs, for timing.

    Mirrors bass2jax.run_bass_via_pjrt's multi-core path, but keeps the
    compiled callable and device inputs so repeated calls measure device
    execution only (no retrace/restage).
    """
    import jax
    from jax.sharding import Mesh, PartitionSpec, NamedSharding
    try:
        from jax.experimental.shard_map import shard_map
    except ImportError:
        from jax.shard_map import shard_map
    from concourse.bass2jax import _bass_exec_p, install_neuronx_cc_hook, partition_id_tensor

    nc = _get_nc()
    install_neuronx_cc_hook()

    partition_name = nc.partition_id_tensor.name if nc.partition_id_tensor else None
    in_names, out_names, out_avals = [], [], []
    for alloc in nc.m.functions[0].allocations:
        if not isinstance(alloc, mybir.MemoryLocationSet):
            continue
        name = alloc.memorylocations[0].name
        if alloc.kind == "ExternalInput":
            if name != partition_name:
                in_names.append(name)
        elif alloc.kind == "ExternalOutput":
            out_avals.append(jax.core.ShapedArray(
                tuple(alloc.tensor_shape), mybir.dt.np(alloc.dtype)))
            out_names.append(name)
    n_params = len(in_names)
    all_in_names = list(in_names) + list(out_names)
    if partition_name is not None:
        all_in_names.append(partition_name)

    def _body(*args):
        operands = list(args)
        if partition_name is not None:
            operands.append(partition_id_tensor())
        return tuple(_bass_exec_p.bind(
            *operands,
            out_avals=tuple(out_avals),
            in_names=tuple(all_in_names),
            out_names=tuple(out_names),
            lowering_input_output_aliases=(),
            sim_require_finite=True,
            sim_require_nnan=True,
            nc=nc,
        ))

    devices = jax.devices()[:N_CORES]
    mesh = Mesh(np.asarray(devices), ("core",))
    n_out = len(out_names)
    sharded = jax.jit(
        shard_map(_body, mesh=mesh,
                  in_specs=(PartitionSpec("core"),) * (n_params + n_out),
                  out_specs=(PartitionSpec("core"),) * n_out,
                  check_rep=False),
        keep_unused=True,
    )
    sh = NamedSharding(mesh, PartitionSpec("core"))
    concat_in = [
        jax.device_put(
            np.concatenate([np.asarray(in_maps[c][k]) for c in range(N_CORES)], 0), sh)
        for k in in_names
    ]
    concat_zeros = [
        jax.device_put(
            np.zeros((N_CORES * a.shape[0], *a.shape[1:]), a.dtype), sh)
        for a in out_avals
    ]
    return sharded, concat_in + concat_zeros, out_names, out_avals
